# revision 1
# baseline (speedup 1.0000x reference)
"""Trainium2 Bass kernel: weighted-KDE avoid-distance (retrieval_knn).

dist[n] = mean_m exp(-0.5 * sum_d (means[m,d]-samples[n,d])^2 / stds[m,d])
out     = -dist + max(dist) + min(dist)

Data-parallel over N=8192 samples across 8 cores (1024 each; full means/stds
per core). Numerics: two bf16 hi/lo matmul passes
  logp[n,m] = sB.mB + s2.w' + a[m]
    w' = -0.5/std, sB = -2s, mB = m*w', s2 = s*s, a[m] = sum_d m^2*w'
  pass1 K=128: lhs [sB_hi s2_hi sB_lo s2_lo] x rhs1 [mB_hi w'_hi mB_hi w'_hi]
    (rhs1 columns duplicated via a stride-0 access pattern on the transpose,
     so only [mB_hi w'_hi] is materialized)
  pass2 K=128: lhs [sB_hi s2_hi ones(64)]    x rhs2 [mB_lo w'_lo mq_hi mq_lo]
    (mq = m^2*w' built from m^2 so the chain starts at means arrival)

Single-shot-latency structure:
  - all operand transposes on the PE; outputs land in bf16 slots carved out
    of the two [128,2048] PSUM main-loop buffers (8 slots = 1 bank; within
    each bank the slot needed first is written last, so a reader never
    overlaps a bank the PE is still writing)
  - dummy PE transposes from t~0.3us ride out the HAM clock-gate ramp
  - a [128,1] exp at t~0 preloads the activation table set off the path
  - 3 parallel DMA channels: stds (SP HWDGE), means (Act HWDGE), samples
    (Pool SWDGE)
  - tiny chunk-0-only copies (s1T/s2T first 128 cols) + per-half rhs
    copies pull the first matmuls ahead of the wide fills
  - exp in full [128,2048] chunks with ScalarE accum_out (chunk 0 in
    halves to start the stream early); remaining transposes interleave
    with chunk-0 matmuls in the in-order PE queue

Final -dist+max+min flip on host after gathering shards.
"""

import sys

import numpy as np

for _p in ("/opt/trn_rl_repo", "/root/.axon_site/_ro/trn_rl_repo"):
    if _p not in sys.path:
        sys.path.insert(0, _p)

N, M, D = 8192, 2048, 32
N_CORES = 8
NSH = N // N_CORES        # 1024 samples per core
MT = M // 128             # 16 mean tiles
CT = NSH // 128           # 8 sample chunks per core
LN_M = float(np.log(M))   # ln(2048); exp bias folds the 1/M mean

N_WARM = 40               # dummy transposes riding out the PE clock ramp

_CACHE = {}


def _build_nc(reps: int = 1):
    import concourse.bacc as bacc
    import concourse.tile as tile
    from concourse import mybir
    from concourse.masks import make_identity

    f32 = mybir.dt.float32
    bf16 = mybir.dt.bfloat16
    AF = mybir.ActivationFunctionType
    OP = mybir.AluOpType
    AX = mybir.AxisListType

    nc = bacc.Bacc("TRN2", target_bir_lowering=False, debug=False)

    samples_d = nc.dram_tensor("samples", [NSH, D], f32, kind="ExternalInput")
    means_d = nc.dram_tensor("means", [M, D], f32, kind="ExternalInput")
    stds_d = nc.dram_tensor("stds", [M, D], f32, kind="ExternalInput")
    dist_d = nc.dram_tensor("dist", [NSH], f32, kind="ExternalOutput")

    with tile.TileContext(nc) as tc:
        with (
            tc.tile_pool(name="persist", bufs=1) as pp,
            tc.tile_pool(name="psum", bufs=2, space="PSUM") as psp,
            tc.tile_pool(name="expo", bufs=3) as xp,
        ):
          for _rep in range(reps):
            # ---- input DMA: stds (SP), means (Act), samples (Pool SWDGE;
            # desc-gen placed after the t0 memsets so the means transfer wins
            # the DMA-engine slot) ----
            stds_nat = pp.tile([128, MT, D], f32)
            means_nat = pp.tile([128, MT, D], f32)
            samp_nat = pp.tile([128, CT, D], f32)
            nc.sync.dma_start(stds_nat[:], stds_d.ap().rearrange("(p t) d -> p t d", p=128))
            nc.scalar.dma_start(means_nat[:], means_d.ap().rearrange("(p t) d -> p t d", p=128))

            # ---- t0 setup ----
            garb = pp.tile([128, 128], bf16)
            nc.vector.memset(garb[:], 0.0)          # warmup operand (DVE, t0)
            scr0 = pp.tile([128, 1], f32)
            nc.vector.memset(scr0[:], 0.0)
            scr1 = pp.tile([128, 1], f32)
            # preload the exp table set while DMA/features run
            nc.scalar.activation(scr1[:], scr0[:], AF.Exp)
            ebias = pp.tile([128, 1], f32)
            nc.vector.memset(ebias[:], -LN_M)
            s2T = pp.tile([128, NSH], bf16)
            nc.vector.memset(s2T[64:128, :], 1.0)   # pass2 ones rows (DVE, idle)
            identity = pp.tile([128, 128], bf16)
            make_identity(nc, identity[:])          # Pool
            nc.gpsimd.dma_start(samp_nat[:], samples_d.ap().rearrange("(p c) d -> p c d", p=128))

            # ---- PSUM slot plan (bf16 slots inside the two mm buffers) ----
            # T0: bank0 s0-7 warmup+sample; bank1 s8-15 rhs1 A;
            #     bank2 s16-23 rhs1 B;      bank3 s24-31 rhs2 A
            # T1: bank4 s0-7 rhs2 B
            T0 = psp.tile([128, M], f32, tag="mm")
            T1 = psp.tile([128, M], f32, tag="mm")
            T0b = T0.bitcast(bf16)
            T1b = T1.bitcast(bf16)

            def slot(tb, k):
                return tb[:, k * 128:(k + 1) * 128]

            for i in range(N_WARM):
                nc.tensor.transpose(slot(T0b, i % 8), garb[:], garb[:])

            # ---- sample features: Act head, DVE lo-tail ----
            # spacked cols: [sB_hi, s2_hi, sB_lo, s2_lo]
            s2 = pp.tile([128, CT, D], f32)
            spacked = pp.tile([128, CT, 128], bf16)
            nc.scalar.mul(spacked[:, :, 0:D], samp_nat[:], -2.0)              # sB_hi
            nc.scalar.activation(s2[:], samp_nat[:], AF.Square)               # s2
            nc.scalar.copy(spacked[:, :, D:2 * D], s2[:])                     # s2_hi
            nc.vector.scalar_tensor_tensor(                                   # sB_lo
                spacked[:, :, 2 * D:3 * D], samp_nat[:], -2.0, spacked[:, :, 0:D],
                op0=OP.mult, op1=OP.subtract)
            nc.vector.scalar_tensor_tensor(                                   # s2_lo
                spacked[:, :, 3 * D:4 * D], s2[:], 1.0, spacked[:, :, D:2 * D],
                op0=OP.mult, op1=OP.subtract)

            # ---- mean features: DVE shallow chain; mq on Pool via m^2 ----
            r = pp.tile([128, MT, D], f32)
            mB = pp.tile([128, MT, D], f32)
            t2 = pp.tile([128, MT, D], f32)
            packed1 = pp.tile([128, MT, 128], bf16)     # [mB_hi w'_hi]x2
            packed2 = pp.tile([128, MT, 128], bf16)     # [mB_lo w'_lo mq_hi mq_lo]
            wf = pp.tile([128, MT, D], f32)
            # stds in [0.5,1.5): safe for the approx path; ~51 ULP on r is
            # ~4e-6 rel, far under budget. Real-HW reciprocal() is ~6 cpe
            # (~3.2us here); this runs in one 1-cpe custom-DVE op.
            nc.vector.reciprocal_approx_fast(r[:], stds_nat[:])
            nc.vector.tensor_scalar_mul(wf[:], r[:], -0.5)                    # w' f32
            nc.vector.tensor_scalar_mul(packed1[:, :, D:2 * D], r[:], -0.5)   # w'_hi
            nc.vector.scalar_tensor_tensor(
                mB[:], means_nat[:], -0.5, r[:], op0=OP.mult, op1=OP.mult)
            nc.vector.tensor_copy(packed1[:, :, 0:D], mB[:])                  # mB_hi
            nc.scalar.copy(packed1[:, :, 2 * D:3 * D], mB[:])                 # mB_hi dup
            nc.scalar.mul(packed1[:, :, 3 * D:4 * D], r[:], -0.5)             # w'_hi dup
            nc.vector.scalar_tensor_tensor(                                   # mB_lo
                packed2[:, :, 0:D], mB[:], 1.0, packed1[:, :, 0:D],
                op0=OP.mult, op1=OP.subtract)
            nc.vector.scalar_tensor_tensor(                                   # w'_lo
                packed2[:, :, D:2 * D], r[:], -0.5, packed1[:, :, D:2 * D],
                op0=OP.mult, op1=OP.subtract)
            m2 = pp.tile([128, MT, D], f32)
            hA = slice(0, MT // 2)
            hB = slice(MT // 2, MT)
            for hh in (hA, hB):
                # Pool-legal ops only (TensorTensor / TensorCopy)
                nc.gpsimd.tensor_mul(m2[:, hh], means_nat[:, hh], means_nat[:, hh])
                nc.gpsimd.tensor_mul(t2[:, hh], m2[:, hh], wf[:, hh])         # m^2*w'
                nc.gpsimd.tensor_copy(packed2[:, hh, 2 * D:3 * D], t2[:, hh])  # mq_hi
                nc.gpsimd.tensor_sub(                                         # mq_lo
                    packed2[:, hh, 3 * D:4 * D], t2[:, hh],
                    packed2[:, hh, 2 * D:3 * D])

            # ---- transposes (PE) + copies ----
            s1T = pp.tile([128, NSH], bf16)
            rhs1 = pp.tile([128, M], bf16)
            rhs2 = pp.tile([128, M], bf16)

            def p1in(t):
                return packed1[:, t, :]

            ORD = [4, 5, 6, 7, 1, 2, 3, 0]           # slot needed first written last
            for c in ORD:                            # sample -> bank0
                nc.tensor.transpose(slot(T0b, c), spacked[:, c, :], identity[:])
            for k in ORD:                            # rhs1 A -> bank1
                nc.tensor.transpose(slot(T0b, 8 + k), p1in(k), identity[:])
            for k in ORD:                            # rhs1 B -> bank2
                nc.tensor.transpose(slot(T0b, 16 + k), p1in(8 + k), identity[:])

            # chunk-0 critical-path copies (DVE); s1T wide fill on Act
            nc.vector.tensor_copy(rhs1[:, 0:1024], T0b[:, 1024:2048])
            nc.vector.tensor_copy(s1T[:, 0:128], slot(T0b, 0))
            nc.vector.tensor_copy(s2T[0:64, 0:128], s1T[0:64, 0:128])
            nc.scalar.copy(s1T[:, 128:1024], T0b[:, 128:1024])
            nc.gpsimd.tensor_copy(s2T[0:64, 128:1024], s1T[0:64, 128:1024])

            # ---- main loop ----
            dist_sb = pp.tile([128, CT], f32)
            dh = pp.tile([128, 2], f32)

            def mm(ps, c, j, pass2):
                sl_ps = ps[:, j * 512:(j + 1) * 512]
                sl_m = slice(j * 512, (j + 1) * 512)
                lhs = (s2T if pass2 else s1T)[:, c * 128:(c + 1) * 128]
                rhs = (rhs2 if pass2 else rhs1)[:, sl_m]
                nc.tensor.matmul(sl_ps, lhsT=lhs, rhs=rhs, start=not pass2,
                                 stop=pass2, skip_group_check=True)

            for c in range(CT):
                ps = psp.tile([128, M], f32, tag="mm")
                # fp32 eo where a DVE reduce does the sum (bf16 would round
                # each exp term by 2^-9 and blow past the error gate)
                if 1 <= c <= 5:
                    eo = xp.tile([128, M], f32, tag="eof", name="eof")
                else:
                    eo = xp.tile([128, M], bf16, tag="eo", name="eo")
                if c == 0:
                    for k in ORD:                    # rhs2 A -> bank3
                        nc.tensor.transpose(slot(T0b, 24 + k), packed2[:, k, :], identity[:])
                    for j in (0, 1):
                        mm(ps, c, j, False)
                    nc.vector.tensor_copy(rhs2[:, 0:1024], T0b[:, 3072:4096])
                    for j in (0, 1):
                        mm(ps, c, j, True)
                    nc.scalar.activation(eo[:, 0:1024], ps[:, 0:1024], AF.Exp,
                                         bias=ebias[:], scale=1.0,
                                         accum_out=dh[:, 0:1])
                    nc.vector.tensor_copy(rhs1[:, 1024:2048], T0b[:, 2048:3072])
                    for j in (2, 3):
                        mm(ps, c, j, False)
                    for k in ORD:                    # rhs2 B -> bank4 (T1)
                        nc.tensor.transpose(slot(T1b, k), packed2[:, 8 + k, :], identity[:])
                    nc.vector.tensor_copy(rhs2[:, 1024:2048], T1b[:, 0:1024])
                    for j in (2, 3):
                        mm(ps, c, j, True)
                    nc.scalar.activation(eo[:, 1024:2048], ps[:, 1024:2048], AF.Exp,
                                         bias=ebias[:], scale=1.0,
                                         accum_out=dh[:, 1:2])
                    nc.vector.tensor_reduce(dist_sb[:, 0:1], dh[:], axis=AX.X,
                                            op=OP.add)
                else:
                    for j in range(4):
                        mm(ps, c, j, False)
                    for j in range(4):
                        mm(ps, c, j, True)
                    if c <= 5:
                        nc.scalar.activation(eo[:], ps[:], AF.Exp, bias=ebias[:],
                                             scale=1.0)
                        nc.vector.tensor_reduce(dist_sb[:, c:c + 1], eo[:],
                                                axis=AX.XY, op=OP.add)
                    else:
                        nc.scalar.activation(eo[:], ps[:], AF.Exp, bias=ebias[:],
                                             scale=1.0, accum_out=dist_sb[:, c:c + 1])

            nc.sync.dma_start(dist_d.ap().rearrange("(p c) -> p c", p=128), dist_sb[:])

    nc.compile()
    return nc


def _get_nc():
    if "nc" not in _CACHE:
        _CACHE["nc"] = _build_nc()
    return _CACHE["nc"]


def kernel(samples: np.ndarray, means: np.ndarray, stds: np.ndarray) -> np.ndarray:
    from concourse.bass_utils import run_bass_kernel_spmd

    samples = np.ascontiguousarray(samples, dtype=np.float32)
    means = np.ascontiguousarray(means, dtype=np.float32)
    stds = np.ascontiguousarray(stds, dtype=np.float32)

    nc = _get_nc()
    in_maps = [
        {"samples": samples[i * NSH:(i + 1) * NSH], "means": means, "stds": stds}
        for i in range(N_CORES)
    ]
    res = run_bass_kernel_spmd(nc, in_maps, list(range(N_CORES)))
    dist = np.concatenate([res.results[i]["dist"] for i in range(N_CORES)])
    return (-dist + dist.max() + dist.min()).astype(np.float32)



# revision 25
# speedup vs baseline: 1.0947x; 1.0947x over previous
"""Trainium2 Bass kernel: weighted-KDE avoid-distance (retrieval_knn).

dist[n] = mean_m exp(-0.5 * sum_d (means[m,d]-samples[n,d])^2 / stds[m,d])
out     = -dist + max(dist) + min(dist)

Data-parallel over N=8192 samples across 8 cores (1024 each; full means/stds
per core).

Single fp32r matmul pass (PE truncates operands to ~fp22; 1 cycle/row at
>=256 moving cols — same rate as bf16, half the passes of the old hi/lo
scheme, and no hi/lo feature packing):
  logp[n,m] = sB.mB + s2.wf + ones.mq      (K = 96 rows of 32)
    sB = -2s (Act), s2 = s^2 (Act Square), ones = 1 (memset at t0)
    r = 1/std (DVE approx), wf = -0.5*r (DVE), mB = m*wf (DVE),
    mq = m*mB = m^2*wf (Pool)
  end-to-end numpy sim of the fp22 truncation: max rel err 5.2e-3 vs the
  2e-2 gate (bf16 single pass would be 0.46; old hi/lo scheme 3.8e-4).

Single-shot-latency structure:
  - stds/means DMA split in halves so the DVE feature chain starts ~0.55us
  - dummy PE transposes from t~0.1us ride out the clock-gate ramp
  - [128,1] exp at t0 preloads the activation table set off the path
  - all 24 operand transposes (8 sample + 16 rhs tiles, f32r 1.5 cyc/row)
    land in f32 slots of the two [128,2048] PSUM main-loop buffers; copies
    to SBUF split across DVE (rhs 0-7), Act (rhs 8-15), Pool (s1T)
  - main loop: 4 fp32r matmuls (512 cols) per chunk; exp in full
    [128,2048] chunks (chunk 0 in halves to start the stream early)
  - reduce split: chunks 0,5,6,7 use ScalarE accum_out (187ns aux each),
    chunks 1-4 fp32 eo + DVE tensor_reduce (DVE idle in main loop)

Final -dist+max+min flip on host after gathering shards.
"""

import sys

import numpy as np

for _p in ("/opt/trn_rl_repo", "/root/.axon_site/_ro/trn_rl_repo"):
    if _p not in sys.path:
        sys.path.insert(0, _p)

N, M, D = 8192, 2048, 32
N_CORES = 8
NSH = N // N_CORES        # 1024 samples per core
MT = M // 128             # 16 mean tiles
CT = NSH // 128           # 8 sample chunks per core
K = 96                    # contraction rows: [sB(32), s2(32), ones(32)]
LN_M = float(np.log(M))   # ln(2048); exp bias folds the 1/M mean

N_WARM = 14               # dummy transposes riding out the PE clock ramp

_CACHE = {}


def _build_nc(reps: int = 1):
    import concourse.bacc as bacc
    import concourse.tile as tile
    from concourse import mybir
    from concourse.masks import make_identity

    f32 = mybir.dt.float32
    f32r = mybir.dt.float32r
    bf16 = mybir.dt.bfloat16
    AF = mybir.ActivationFunctionType
    OP = mybir.AluOpType
    AX = mybir.AxisListType

    nc = bacc.Bacc("TRN2", target_bir_lowering=False, debug=False)

    samples_d = nc.dram_tensor("samples", [NSH, D], f32, kind="ExternalInput")
    means_d = nc.dram_tensor("means", [M, D], f32, kind="ExternalInput")
    stds_d = nc.dram_tensor("stds", [M, D], f32, kind="ExternalInput")
    dist_d = nc.dram_tensor("dist", [NSH], f32, kind="ExternalOutput")

    with tile.TileContext(nc) as tc:
        with (
            tc.tile_pool(name="persist", bufs=1) as pp,
            tc.tile_pool(name="psum", bufs=2, space="PSUM") as psp,
            tc.tile_pool(name="expo", bufs=3) as xp,
        ):
          for _rep in range(reps):
            # ---- t0: PE warmup operand first so the clock ramp starts asap
            garb = pp.tile([128, 128], bf16)
            nc.vector.memset(garb[:], 0.0)
            # input DMA: stds (SP), means (Act) unsplit — HWDGE descriptor
            # processing is ~625ns per transfer regardless of size and fully
            # serialized, so fewer transfers reach the last-needed tensor
            # (means) sooner. Samples ride the Pool SWDGE (separate desc
            # path), chunk 0 split out so its lhs chain starts early.
            stds_nat = pp.tile([128, MT, D], f32)
            means_nat = pp.tile([128, MT, D], f32)
            samp_c0 = pp.tile([128, 1, D], f32)
            samp_nat = pp.tile([128, CT - 1, D], f32)
            stds_ap = stds_d.ap().rearrange("(p t) d -> p t d", p=128)
            means_ap = means_d.ap().rearrange("(p t) d -> p t d", p=128)
            samp_ap = samples_d.ap().rearrange("(p c) d -> p c d", p=128)
            nc.sync.dma_start(stds_nat[:], stds_ap[:])
            nc.scalar.dma_start(means_nat[:], means_ap[:])
            nc.gpsimd.dma_start(samp_c0[:], samp_ap[:, 0:1])
            nc.gpsimd.dma_start(samp_nat[:], samp_ap[:, 1:CT])

            scr0 = pp.tile([128, 1], f32)
            nc.vector.memset(scr0[:], 0.0)
            scr1 = pp.tile([128, 1], f32)
            # preload the exp table set while DMA/features run
            nc.scalar.activation(scr1[:], scr0[:], AF.Exp)
            ebias = pp.tile([128, 1], f32)
            nc.vector.memset(ebias[:], -LN_M)
            spack0 = pp.tile([128, 1, K], f32)
            spackR = pp.tile([128, CT - 1, K], f32)
            nc.vector.memset(spack0[:, :, 64:96], 1.0)    # ones rows
            nc.vector.memset(spackR[:, :, 64:96], 1.0)
            identity = pp.tile([128, 128], f32)
            make_identity(nc, identity[:])                # Pool

            # ---- PSUM slot plan (f32 [*,128] slots in the two mm buffers)
            # T1 slots 0-7: rhs tiles 0-7; T1 slots 8-15: sample tiles 0-7
            # T0 slots 8-15: rhs tiles 8-15
            T0 = psp.tile([128, M], f32, tag="mm")
            T1 = psp.tile([128, M], f32, tag="mm")

            def slot(tb, k):
                return tb[:, k * 128:(k + 1) * 128]

            T0b = T0.bitcast(bf16)
            for i in range(N_WARM):
                nc.tensor.transpose(T0b[:, (i % 8) * 128:(i % 8) * 128 + 128], garb[:], garb[:])

            # ---- features ----
            # samples (Act): sB = -2s, s2 = s^2; chunk 0 split out so its
            # transpose can go as soon as the tiny chunk-0 DMA lands
            nc.scalar.mul(spack0[:, :, 0:D], samp_c0[:], -2.0)
            nc.scalar.activation(spack0[:, :, D:2 * D], samp_c0[:], AF.Square)
            nc.scalar.mul(spackR[:, :, 0:D], samp_nat[:], -2.0)
            nc.scalar.activation(spackR[:, :, D:2 * D], samp_nat[:], AF.Square)
            # means/stds: DVE chain recip -> wf -> mB -> mq
            r = pp.tile([128, MT, D], f32)
            packed1 = pp.tile([128, MT, K], f32)
            nc.vector.reciprocal_approx_fast(r[:], stds_nat[:])
            nc.vector.tensor_scalar_mul(packed1[:, :, D:2 * D], r[:], -0.5)
            nc.vector.scalar_tensor_tensor(                           # mB = -0.5*m*r
                packed1[:, :, 0:D], means_nat[:], -0.5, r[:],
                op0=OP.mult, op1=OP.mult)
            nc.vector.scalar_tensor_tensor(                           # mq = m*mB
                packed1[:, :, 2 * D:3 * D], means_nat[:], 1.0,
                packed1[:, :, 0:D], op0=OP.mult, op1=OP.mult)

            # ---- transposes (PE, f32r 1.5 cyc/row) + copies ----
            # Slot plan (dependency tracking is tile-granular, so staging is
            # arranged to keep false WARs off the chunk-0 matmul path):
            #   T0 slots 8-15: sample tiles 0-7 (drained by Pool s1T copies)
            #   T1 slots 0-15: rhs tiles 0-15 (drained by DVE/Act copies)
            # Uniform chunks afterwards — no early-exp special case; the WAR
            # chains of interleaved schemes cost more than they save.
            s1T = pp.tile([128, NSH], f32r)
            rhs01 = pp.tile([128, 1024], f32r)
            rhs23 = pp.tile([128, 1024], f32r)

            def tp(dst_slot, src):
                nc.tensor.transpose(dst_slot[0:K], src, identity[:])

            dist_sb = pp.tile([128, CT], f32)

            def mm(ps, c, j):
                rtile = rhs01 if j < 2 else rhs23
                nc.tensor.matmul(
                    ps[:, j * 512:(j + 1) * 512],
                    lhsT=s1T[0:K, c * 128:(c + 1) * 128],
                    rhs=rtile[0:K, (j % 2) * 512:(j % 2) * 512 + 512],
                    start=True, stop=True, skip_group_check=True)

            tp(slot(T0, 8), spack0[:, 0, :])             # sample tile 0 first
            nc.vector.tensor_copy(s1T[0:K, 0:128], T0[0:K, 1024:1152])
            for c in range(1, CT):                       # sample tiles 1-7 -> T0 s9-15
                tp(slot(T0, 8 + c), spackR[:, c - 1, :])
            nc.scalar.copy(s1T[0:K, 128:1024], T0[0:K, 1152:2048])
            for t in range(MT):                          # rhs tiles 0-15 -> T1
                tp(slot(T1, t), packed1[:, t, :])
            # copies read T1 after all transposes land
            nc.vector.tensor_copy(rhs01[0:K, :], T1[0:K, 0:1024])
            nc.vector.tensor_copy(rhs23[0:K, :], T1[0:K, 1024:2048])

            # ---- main loop; chunk 0 split in halves so its first exp
            # starts as soon as rhs01 lands (the per-tile access chain
            # serializes mm/exp interleave safely in program order) ----
            dh = pp.tile([128, 2], f32)
            for c in range(CT):
                ps = psp.tile([128, M], f32, tag="mm")
                if c <= 4:
                    eo = xp.tile([128, M], f32, tag="eof", name="eof")
                else:
                    eo = xp.tile([128, M], bf16, tag="eo", name="eo")
                if c == 0:
                    mm(ps, c, 0)
                    mm(ps, c, 1)
                    nc.scalar.activation(eo[:, 0:1024], ps[:, 0:1024], AF.Exp,
                                         bias=ebias[:], scale=1.0)
                    mm(ps, c, 2)
                    mm(ps, c, 3)
                    nc.scalar.activation(eo[:, 1024:2048], ps[:, 1024:2048],
                                         AF.Exp, bias=ebias[:], scale=1.0)
                    nc.vector.tensor_reduce(dh[:, 0:1], eo[:, 0:1024],
                                            axis=AX.X, op=OP.add)
                    nc.vector.tensor_reduce(dh[:, 1:2], eo[:, 1024:2048],
                                            axis=AX.X, op=OP.add)
                    nc.vector.tensor_reduce(dist_sb[:, 0:1], dh[:], axis=AX.X,
                                            op=OP.add)
                    continue
                for j in range(4):
                    mm(ps, c, j)
                if c <= 4:
                    nc.scalar.activation(eo[:], ps[:], AF.Exp, bias=ebias[:],
                                         scale=1.0)
                    nc.vector.tensor_reduce(dist_sb[:, c:c + 1], eo[:],
                                            axis=AX.XY, op=OP.add)
                else:
                    nc.scalar.activation(eo[:], ps[:], AF.Exp, bias=ebias[:],
                                         scale=1.0, accum_out=dist_sb[:, c:c + 1])

            # output DMA split on the Pool SWDGE: descriptor generation
            # pre-runs during the exp stream (HWDGE descs can't)
            dist_ap = dist_d.ap().rearrange("(p c) -> p c", p=128)
            nc.gpsimd.dma_start(dist_ap[:, 0:CT - 1], dist_sb[:, 0:CT - 1])
            nc.gpsimd.dma_start(dist_ap[:, CT - 1:CT], dist_sb[:, CT - 1:CT])

    nc.compile()
    return nc


def _get_nc():
    if "nc" not in _CACHE:
        _CACHE["nc"] = _build_nc()
    return _CACHE["nc"]


def kernel(samples: np.ndarray, means: np.ndarray, stds: np.ndarray) -> np.ndarray:
    from concourse.bass_utils import run_bass_kernel_spmd

    samples = np.ascontiguousarray(samples, dtype=np.float32)
    means = np.ascontiguousarray(means, dtype=np.float32)
    stds = np.ascontiguousarray(stds, dtype=np.float32)

    nc = _get_nc()
    in_maps = [
        {"samples": samples[i * NSH:(i + 1) * NSH], "means": means, "stds": stds}
        for i in range(N_CORES)
    ]
    res = run_bass_kernel_spmd(nc, in_maps, list(range(N_CORES)))
    dist = np.concatenate([res.results[i]["dist"] for i in range(N_CORES)])
    return (-dist + dist.max() + dist.min()).astype(np.float32)


# revision 33
# speedup vs baseline: 2.2375x; 2.0440x over previous
"""Trainium2 Bass kernel: weighted-KDE avoid-distance (retrieval_knn).

dist[n] = mean_m exp(-0.5 * sum_d (means[m,d]-samples[n,d])^2 / stds[m,d])
out     = -dist + max(dist) + min(dist)

Data-parallel over N=8192 samples across 8 cores (1024 each; full means/stds
per core).

Single fp32r matmul pass (PE rounds operands to ~fp22; 1 cycle/row at
>=256 moving cols — same rate as bf16, HALF the passes of the old bf16
hi/lo scheme, and no hi/lo feature packing):
  logp[n,m] = sB.mB + s2.wf + ones.mq      (K = 96 rows of 32)
    sB = -2s (Act), s2 = s^2 (Act Square), ones = 1 (memset at t0)
    r = 1/std (DVE approx recip), wf = -0.5r, mB = m*wf, mq = m*mB (DVE)
  walrus requires fp32r matmul inputs to be *produced* as fp32r, so the
  PSUM->SBUF staging copies write f32r tiles (the copy rounds); the
  transposes stay plain f32.  Measured HW rel err 2.1e-3 vs the 2e-2
  gate (numpy sim of fp22 truncation: 5.2e-3; single-pass bf16: 0.46).

Single-shot-latency structure (timed against TimelineSim, whose
per-tile access chains and DMA latency model match the scheduler):
  - stds (SP) / means (Act) DMA'd unsplit: HWDGE descriptor processing
    is ~625ns per transfer and serialized, so fewer transfers reach the
    last-needed tensor sooner; each DMA pays desc(625) + DGE(650) +
    transfer + 900ns semaphore propagation
  - samples ride the Pool SWDGE (separate descriptor path), chunk 0
    split out so the chunk-0 lhs transpose can go early
  - dummy PE transposes from t~1us ride out the clock-gate ramp
  - [128,1] exp at t0 preloads the activation table set off the path
  - feature chain on DVE (recip -> wf -> mB -> mq), sample features on
    Act; all 24 transposes (f32, PE) land in PSUM slots: samples in T0
    cols 1024:2048, rhs tiles in T1; drains: s1T (DVE+Act), rhs01
    (DVE), rhs23 (Act).  Scheduling rule: every access to a tile
    serializes with all other accesses to that tile (read-read too), so
    staging keeps per-tile chains short and puts copies after all
    transposes of their source tile
  - main loop: 4 fp32r matmuls (512 cols) per chunk; exp in full
    [128,2048] chunks; chunk 0 split in halves so its first exp starts
    as soon as rhs01 lands
  - reduce split: chunks 5-7 use ScalarE accum_out (187ns aux each),
    chunks 0-4 fp32 eo + DVE tensor_reduce (DVE is idle in main loop;
    GpSimd cannot access PSUM so Pool stays off the drain/reduce paths)
  - output DMA split on SP: bulk piece [*,0:7] goes while chunk 7
    finishes, final element alone pays the post-transfer latency

Final -dist+max+min flip on host after gathering shards.
"""

import sys

import numpy as np

for _p in ("/opt/trn_rl_repo", "/root/.axon_site/_ro/trn_rl_repo"):
    if _p not in sys.path:
        sys.path.insert(0, _p)

N, M, D = 8192, 2048, 32
N_CORES = 8
NSH = N // N_CORES        # 1024 samples per core
MT = M // 128             # 16 mean tiles
CT = NSH // 128           # 8 sample chunks per core
K = 96                    # contraction rows: [sB(32), s2(32), ones(32)]
LN_M = float(np.log(M))   # ln(2048); exp bias folds the 1/M mean

N_WARM = 14               # dummy transposes riding out the PE clock ramp

_CACHE = {}


def _build_nc(reps: int = 1):
    import concourse.bacc as bacc
    import concourse.tile as tile
    from concourse import mybir
    from concourse.masks import make_identity

    f32 = mybir.dt.float32
    f32r = mybir.dt.float32r
    bf16 = mybir.dt.bfloat16
    AF = mybir.ActivationFunctionType
    OP = mybir.AluOpType
    AX = mybir.AxisListType

    nc = bacc.Bacc("TRN2", target_bir_lowering=False, debug=False)

    samples_d = nc.dram_tensor("samples", [NSH, D], f32, kind="ExternalInput")
    means_d = nc.dram_tensor("means", [M, D], f32, kind="ExternalInput")
    stds_d = nc.dram_tensor("stds", [M, D], f32, kind="ExternalInput")
    dist_d = nc.dram_tensor("dist", [NSH], f32, kind="ExternalOutput")

    with tile.TileContext(nc) as tc:
        with (
            tc.tile_pool(name="persist", bufs=1) as pp,
            tc.tile_pool(name="psum", bufs=2, space="PSUM") as psp,
            tc.tile_pool(name="expo", bufs=3) as xp,
        ):
          for _rep in range(reps):
            # ---- t0: PE warmup operand first so the clock ramp starts asap
            garb = pp.tile([128, 128], bf16)
            nc.vector.memset(garb[:], 0.0)
            # input DMA: stds (SP), means (Act) unsplit — HWDGE descriptor
            # processing is ~625ns per transfer regardless of size and fully
            # serialized, so fewer transfers reach the last-needed tensor
            # (means) sooner. Samples ride the Pool SWDGE (separate desc
            # path), chunk 0 split out so its lhs chain starts early.
            stds_nat = pp.tile([128, MT, D], f32)
            means_nat = pp.tile([128, MT, D], f32)
            samp_c0 = pp.tile([128, 1, D], f32)
            samp_nat = pp.tile([128, CT - 1, D], f32)
            stds_ap = stds_d.ap().rearrange("(p t) d -> p t d", p=128)
            means_ap = means_d.ap().rearrange("(p t) d -> p t d", p=128)
            samp_ap = samples_d.ap().rearrange("(p c) d -> p c d", p=128)
            nc.sync.dma_start(stds_nat[:], stds_ap[:])
            nc.scalar.dma_start(means_nat[:], means_ap[:])
            nc.gpsimd.dma_start(samp_c0[:], samp_ap[:, 0:1])
            nc.gpsimd.dma_start(samp_nat[:], samp_ap[:, 1:CT])

            scr0 = pp.tile([128, 1], f32)
            nc.vector.memset(scr0[:], 0.0)
            scr1 = pp.tile([128, 1], f32)
            # preload the exp table set while DMA/features run
            nc.scalar.activation(scr1[:], scr0[:], AF.Exp)
            ebias = pp.tile([128, 1], f32)
            nc.vector.memset(ebias[:], -LN_M)
            spack0 = pp.tile([128, 1, K], f32)
            spackR = pp.tile([128, CT - 1, K], f32)
            nc.vector.memset(spack0[:, :, 64:96], 1.0)    # ones rows
            nc.vector.memset(spackR[:, :, 64:96], 1.0)
            identity = pp.tile([128, 128], f32)
            make_identity(nc, identity[:])                # Pool

            # ---- PSUM slot plan (f32 [*,128] slots in the two mm buffers)
            # T1 slots 0-7: rhs tiles 0-7; T1 slots 8-15: sample tiles 0-7
            # T0 slots 8-15: rhs tiles 8-15
            T0 = psp.tile([128, M], f32, tag="mm")
            T1 = psp.tile([128, M], f32, tag="mm")

            def slot(tb, k):
                return tb[:, k * 128:(k + 1) * 128]

            T0b = T0.bitcast(bf16)
            for i in range(N_WARM):
                nc.tensor.transpose(T0b[:, (i % 8) * 128:(i % 8) * 128 + 128], garb[:], garb[:])

            # ---- features ----
            # samples (Act): sB = -2s, s2 = s^2; chunk 0 split out so its
            # transpose can go as soon as the tiny chunk-0 DMA lands
            nc.scalar.mul(spack0[:, :, 0:D], samp_c0[:], -2.0)
            nc.scalar.activation(spack0[:, :, D:2 * D], samp_c0[:], AF.Square)

            # means/stds feature chain on DVE: recip -> wf -> mB -> mq
            r = pp.tile([128, MT, D], f32)
            packed1 = pp.tile([128, MT, K], f32)
            nc.vector.reciprocal_approx_fast(r[:], stds_nat[:])
            nc.vector.tensor_scalar_mul(packed1[:, :, D:2 * D], r[:], -0.5)
            nc.scalar.mul(spackR[:, :, 0:D], samp_nat[:], -2.0)
            nc.scalar.activation(spackR[:, :, D:2 * D], samp_nat[:], AF.Square)
            nc.vector.scalar_tensor_tensor(                           # mB = -0.5*m*r
                packed1[:, :, 0:D], means_nat[:], -0.5, r[:],
                op0=OP.mult, op1=OP.mult)
            nc.vector.scalar_tensor_tensor(                           # mq = m*mB
                packed1[:, :, 2 * D:3 * D], means_nat[:], 1.0,
                packed1[:, :, 0:D], op0=OP.mult, op1=OP.mult)

            # ---- transposes (PE, f32) + staging copies ----
            # Slot plan (every access to a tile serializes with all other
            # accesses to it, so per-tile chains are kept short and copies
            # come after all transposes of their source tile):
            #   T0 slots 8-15: sample tiles 0-7 (drained by DVE+Act s1T copies)
            #   T1 slots 0-15: rhs tiles 0-15 (drained by DVE+Act copies)
            # The copies write f32r tiles — walrus requires fp32r matmul
            # inputs to be produced as fp32r (the copy does the rounding).
            s1T = pp.tile([128, NSH], f32r)
            rhs01 = pp.tile([128, 1024], f32r)
            rhs23 = pp.tile([128, 1024], f32r)

            def tp(dst_slot, src):
                nc.tensor.transpose(dst_slot[0:K], src, identity[:])

            dist_sb = pp.tile([128, CT], f32)

            def mm(ps, c, j):
                rtile = rhs01 if j < 2 else rhs23
                nc.tensor.matmul(
                    ps[:, j * 512:(j + 1) * 512],
                    lhsT=s1T[0:K, c * 128:(c + 1) * 128],
                    rhs=rtile[0:K, (j % 2) * 512:(j % 2) * 512 + 512],
                    start=True, stop=True, skip_group_check=True)

            tp(slot(T0, 8), spack0[:, 0, :])             # sample tile 0 first
            nc.vector.tensor_copy(s1T[0:K, 0:128], T0[0:K, 1024:1152])
            for c in range(1, CT):                       # sample tiles 1-7 -> T0 s9-15
                tp(slot(T0, 8 + c), spackR[:, c - 1, :])
            nc.scalar.copy(s1T[0:K, 128:1024], T0[0:K, 1152:2048])
            for t in range(MT):                          # rhs tiles 0-15 -> T1
                tp(slot(T1, t), packed1[:, t, :])
            # copies read T1 after all transposes land: rhs01 on DVE,
            # rhs23 on Act (fills its idle window before the exp stream)
            nc.vector.tensor_copy(rhs01[0:K, :], T1[0:K, 0:1024])
            nc.scalar.copy(rhs23[0:K, :], T1[0:K, 1024:2048])

            # ---- main loop; chunk 0 split in halves so its first exp
            # starts as soon as rhs01 lands (the per-tile access chain
            # serializes mm/exp interleave safely in program order) ----
            dh = pp.tile([128, 2], f32)
            for c in range(CT):
                ps = psp.tile([128, M], f32, tag="mm")
                if c <= 4:
                    eo = xp.tile([128, M], f32, tag="eof", name="eof")
                else:
                    eo = xp.tile([128, M], bf16, tag="eo", name="eo")
                if c == 0:
                    mm(ps, c, 0)
                    mm(ps, c, 1)
                    nc.scalar.activation(eo[:, 0:1024], ps[:, 0:1024], AF.Exp,
                                         bias=ebias[:], scale=1.0)
                    mm(ps, c, 2)
                    mm(ps, c, 3)
                    nc.scalar.activation(eo[:, 1024:2048], ps[:, 1024:2048],
                                         AF.Exp, bias=ebias[:], scale=1.0)
                    nc.vector.tensor_reduce(dh[:, 0:1], eo[:, 0:1024],
                                            axis=AX.X, op=OP.add)
                    nc.vector.tensor_reduce(dh[:, 1:2], eo[:, 1024:2048],
                                            axis=AX.X, op=OP.add)
                    nc.vector.tensor_reduce(dist_sb[:, 0:1], dh[:], axis=AX.X,
                                            op=OP.add)
                    continue
                for j in range(4):
                    mm(ps, c, j)
                if c <= 4:
                    nc.scalar.activation(eo[:], ps[:], AF.Exp, bias=ebias[:],
                                         scale=1.0)
                    nc.vector.tensor_reduce(dist_sb[:, c:c + 1], eo[:],
                                            axis=AX.XY, op=OP.add)
                else:
                    nc.scalar.activation(eo[:], ps[:], AF.Exp, bias=ebias[:],
                                         scale=1.0, accum_out=dist_sb[:, c:c + 1])

            # output DMA split on SP: the bulk piece overlaps chunk 7's exp,
            # only the final element pays the post-transfer latency
            dist_ap = dist_d.ap().rearrange("(p c) -> p c", p=128)
            nc.sync.dma_start(dist_ap[:, 0:CT - 1], dist_sb[:, 0:CT - 1])
            nc.sync.dma_start(dist_ap[:, CT - 1:CT], dist_sb[:, CT - 1:CT])

    nc.compile()
    return nc


def _get_nc():
    if "nc" not in _CACHE:
        _CACHE["nc"] = _build_nc()
    return _CACHE["nc"]


def kernel(samples: np.ndarray, means: np.ndarray, stds: np.ndarray) -> np.ndarray:
    from concourse.bass_utils import run_bass_kernel_spmd

    samples = np.ascontiguousarray(samples, dtype=np.float32)
    means = np.ascontiguousarray(means, dtype=np.float32)
    stds = np.ascontiguousarray(stds, dtype=np.float32)

    nc = _get_nc()
    in_maps = [
        {"samples": samples[i * NSH:(i + 1) * NSH], "means": means, "stds": stds}
        for i in range(N_CORES)
    ]
    res = run_bass_kernel_spmd(nc, in_maps, list(range(N_CORES)))
    dist = np.concatenate([res.results[i]["dist"] for i in range(N_CORES)])
    return (-dist + dist.max() + dist.min()).astype(np.float32)
